# revision 9
# baseline (speedup 1.0000x reference)
"""Trainium2 Bass kernel for nn_Detector_head (conv-conv-pixelshuffle-NMS).

Self-contained: hardcodes shapes for the 32x128x60x80 detector-head problem,
shards the batch over 8 NeuronCores (4 images each), and reproduces
reference.reference():
  logits  = bn2(conv1x1(relu(bn1(conv3x3(x)))))         [32, 65, 60, 80]
  prob    = pixel_shuffle(logits[:, :64], 8)            [32, 480, 640]
  prob_nms= top1000(local-max-9x9(prob))                [32, 480, 640]
  pred    = (prob_nms >= 0.015).int32                   [32, 480, 640]

Convolutions run as 3-term scale-balanced fp16 splits on the PE
(hi@hi + (hi_w/256)@lo_x + lo_w@(hi_x/256), lo := (v - fp16(v))*256),
measured ~2e-7 relative error vs fp32 on hardware. All NMS comparisons,
the exact top-1000 threshold (arithmetic multi-round counting) and the
final thresholds run in fp32.
"""
import json
import os
import sys

sys.path.insert(0, "/opt/trn_rl_repo")

import numpy as np

import concourse.bass as bass
import concourse.mybir as mybir
import concourse.tile as tile
from concourse.bass_utils import run_bass_kernel_spmd

F32 = mybir.dt.float32
F16 = mybir.dt.float16
BF16 = mybir.dt.bfloat16
I32 = mybir.dt.int32
AT = mybir.ActivationFunctionType
OP = mybir.AluOpType
AX = mybir.AxisListType

N_CORES = 8
B = 4                 # images per core
CIN, CHID, COUT = 128, 256, 65
H, W = 60, 80
HW = H * W
HP, WP = 62, 82       # padded input dims
Y, X = 480, 640
XT = 656              # NMS tile width: 8 pad | 640 | 8 pad
NT = 10               # n-tiles per image
NTP = 480             # pixels per n-tile (6 h-rows x 80)
HROWS = 6
CHUNKS = 4            # 120 core rows + 4 halo rows each side
NEG = -1.0e30
DET = 0.015
TOPK = 1000
NSEL_ROUNDS = 6
TSEL = 32

_KERNEL_CACHE = {}


# --------------------------------------------------------------------------
# Walrus in this toolchain rejects >1 sync-wait per instruction; Tile emits
# multi-wait instructions freely. Split excess waits onto same-engine Drain
# carriers inserted just before the offending instruction.
# --------------------------------------------------------------------------
def _split_sync_waits(js: bytes, max_waits: int = 1) -> bytes:
    mod = json.loads(js)
    for f in mod.get("functions", []):
        for blk in f.get("blocks", []):
            out = []
            for inst in blk.get("instructions", []):
                si = inst.get("sync_info")
                ow = (si or {}).get("on_wait") or []
                if len(ow) > max_waits:
                    extra = ow[: len(ow) - max_waits]
                    si["on_wait"] = ow[len(ow) - max_waits:]
                    for j in range(0, len(extra), max_waits):
                        out.append({
                            "debug": inst.get("debug", 0),
                            "engine": inst["engine"],
                            "ins": [], "outs": [],
                            "name": f"{inst['name']}-wsplit{j}",
                            "opcode": "Drain",
                            "sync_info": {"on_update": [],
                                          "on_wait": extra[j:j + max_waits]},
                        })
                out.append(inst)
            blk["instructions"] = out
    return json.dumps(mod).encode()


def _patch_nc(nc):
    orig = nc.to_json_bytes
    nc.to_json_bytes = lambda *a, **kw: _split_sync_waits(orig(*a, **kw))
    return nc


def _split_fp16(a):
    hi = a.astype(np.float16)
    lo = ((a.astype(np.float32) - hi.astype(np.float32)) * 256.0).astype(np.float16)
    hi2 = (hi.astype(np.float32) / 256.0).astype(np.float16)
    return hi, hi2, lo


def build_kernel():
    nc = bass.Bass("TRN2", target_bir_lowering=False, debug=False,
                   num_devices=1)

    x_in = nc.dram_tensor("x", [B, CIN, H, W], F32, kind="ExternalInput").ap()
    w1h_in = nc.dram_tensor("w1h", [9, CIN, CHID], F16, kind="ExternalInput").ap()
    w1h2_in = nc.dram_tensor("w1h2", [9, CIN, CHID], F16, kind="ExternalInput").ap()
    w1l_in = nc.dram_tensor("w1l", [9, CIN, CHID], F16, kind="ExternalInput").ap()
    w2h_in = nc.dram_tensor("w2h", [2, 128, COUT], F16, kind="ExternalInput").ap()
    w2h2_in = nc.dram_tensor("w2h2", [2, 128, COUT], F16, kind="ExternalInput").ap()
    w2l_in = nc.dram_tensor("w2l", [2, 128, COUT], F16, kind="ExternalInput").ap()
    sb1_in = nc.dram_tensor("sb1", [2, CHID], F32, kind="ExternalInput").ap()
    sb2_in = nc.dram_tensor("sb2", [2, COUT], F32, kind="ExternalInput").ap()
    frac_in = nc.dram_tensor("frac", [1, 4 * TSEL], F32, kind="ExternalInput").ap()

    logits_out = nc.dram_tensor("logits", [B, COUT, H, W], F32,
                                kind="ExternalOutput").ap()
    prob_out = nc.dram_tensor("prob", [B, Y, X], F32, kind="ExternalOutput").ap()
    pnms_out = nc.dram_tensor("pnms", [B, Y, X], F32, kind="ExternalOutput").ap()
    pred_out = nc.dram_tensor("pred", [B, Y, X], I32, kind="ExternalOutput").ap()

    with tile.TileContext(nc) as tc:
        _body(nc, tc, x_in,
              (w1h_in, w1h2_in, w1l_in), (w2h_in, w2h2_in, w2l_in),
              sb1_in, sb2_in, frac_in,
              logits_out, prob_out, pnms_out, pred_out)

    return _patch_nc(nc)


def _body(nc, tc, x_in, w1_ins, w2_ins, sb1_in, sb2_in, frac_in,
          logits_out, prob_out, pnms_out, pred_out):
    from contextlib import ExitStack
    ctx = ExitStack()
    consts = ctx.enter_context(tc.tile_pool(name="consts", bufs=1))
    wpool = ctx.enter_context(tc.tile_pool(name="w", bufs=1))
    xpool = ctx.enter_context(tc.tile_pool(name="xp", bufs=1))
    xsplit = ctx.enter_context(tc.tile_pool(name="xsplit", bufs=2))
    h2pool = ctx.enter_context(tc.tile_pool(name="h2", bufs=2))
    lgpool = ctx.enter_context(tc.tile_pool(name="lg", bufs=2))
    nmswork = ctx.enter_context(tc.tile_pool(name="nmsw", bufs=7))
    keeppool = ctx.enter_context(tc.tile_pool(name="keep", bufs=16))
    outpool = ctx.enter_context(tc.tile_pool(name="outp", bufs=3))
    bmpool = ctx.enter_context(tc.tile_pool(name="bm", bufs=2))
    selpool = ctx.enter_context(tc.tile_pool(name="sel", bufs=1))
    smpool = ctx.enter_context(tc.tile_pool(name="sm", bufs=2))
    ps1 = ctx.enter_context(tc.tile_pool(name="ps1", bufs=3, space="PSUM"))
    ps2 = ctx.enter_context(tc.tile_pool(name="ps2", bufs=2, space="PSUM"))
    psb = ctx.enter_context(tc.tile_pool(name="psb", bufs=1, space="PSUM"))

    # ---------------- weights / constants ----------------
    w1 = []
    for nm, src in zip(("w1h", "w1h2", "w1l"), w1_ins):
        t = wpool.tile([CIN, 9 * CHID], F16, tag=nm)
        nc.sync.dma_start(t[:].rearrange("k (t m) -> k t m", t=9),
                          src.rearrange("t k m -> k t m"))
        w1.append(t)
    w2 = []
    for nm, src in zip(("w2h", "w2h2", "w2l"), w2_ins):
        t = wpool.tile([128, 2 * COUT], F16, tag=nm)
        nc.sync.dma_start(t[:].rearrange("k (t m) -> k t m", t=2),
                          src.rearrange("t k m -> k t m"))
        w2.append(t)

    s1 = consts.tile([128, 4], F32, tag="s1")   # [:,m]=scale1_m ; [:,2+m]=scale1_m*256
    b1 = consts.tile([128, 4], F32, tag="b1")
    for m in range(2):
        nc.sync.dma_start(
            s1[:, m:m + 1],
            sb1_in[0:1, m * 128:(m + 1) * 128].rearrange("a k -> k a"))
        nc.sync.dma_start(
            b1[:, m:m + 1],
            sb1_in[1:2, m * 128:(m + 1) * 128].rearrange("a k -> k a"))
    nc.vector.tensor_scalar_mul(s1[:, 2:4], s1[:, 0:2], 256.0)
    nc.vector.tensor_scalar_mul(b1[:, 2:4], b1[:, 0:2], 256.0)
    s2 = consts.tile([COUT, 1], F32, tag="s2")
    nc.sync.dma_start(s2[:], sb2_in[0:1, :].rearrange("a k -> k a"))
    b2 = consts.tile([COUT, 1], F32, tag="b2")
    nc.sync.dma_start(b2[:], sb2_in[1:2, :].rearrange("a k -> k a"))

    frac = consts.tile([1, 4 * TSEL], F32, tag="frac")
    nc.sync.dma_start(frac[:], frac_in)
    ones96 = consts.tile([1, 96], F32, tag="ones96")
    nc.vector.memset(ones96[:], 1.0)
    ones128 = consts.tile([1, 128], F32, tag="ones128")
    nc.vector.memset(ones128[:], 1.0)

    m_all = consts.tile([96, 128], F32, tag="m_all")
    lo4 = consts.tile([1, 4], F32, tag="lo4")
    hi4 = consts.tile([1, 4], F32, tag="hi4")
    nc.vector.memset(lo4[:], 0.0)
    nc.vector.memset(hi4[:], 16.0)
    thr_all = consts.tile([128, 4], F32, tag="thr_all")
    cand = []
    for bb in range(B):
        cnd = consts.tile([96, 128], F32, tag=f"cand{bb}", name=f"cand{bb}")
        cand.append(cnd)

    keep_tiles = {}

    # =====================================================================
    # per-image conv + NMS chunks
    # =====================================================================
    for b in range(B):
        xpad = xpool.tile([CIN, HP * WP], F32, tag="xpad")
        nc.vector.memset(xpad[:], 0.0)
        nc.sync.dma_start(
            xpad[:].rearrange("k (h w) -> k h w", h=HP, w=WP)[:, 1:61, 1:81],
            x_in[b])

        lg = lgpool.tile([COUT, HW], F32, tag="lg")

        for nt in range(NT):
            h0 = nt * HROWS
            # ---- per-ntile fp16 split of 8 padded rows [h0, h0+8) ----
            xsl = xpad[:].rearrange("k (h w) -> k h w", h=HP, w=WP)[:, h0:h0 + 8, :]
            xsl = xsl.rearrange("k h w -> k (h w)")
            xh = xsplit.tile([CIN, 8 * WP], F16, tag="xh")
            nc.scalar.copy(xh[:], xsl)
            xtmp = xsplit.tile([CIN, 8 * WP], F32, tag="xtmp")
            nc.scalar.mul(xtmp[:], xsl, 256.0)
            xl = xsplit.tile([CIN, 8 * WP], F16, tag="xl")
            nc.vector.scalar_tensor_tensor(xl[:], xh[:], -256.0, xtmp[:],
                                           op0=OP.mult, op1=OP.add)
            xh2 = xsplit.tile([CIN, 8 * WP], F16, tag="xh2")
            nc.gpsimd.tensor_scalar_mul(xh2[:], xh[:], 1.0 / 256.0)

            def mk_rhs(t, dy, dx):
                v = t[:].rearrange("k (h w) -> k h w", h=8, w=WP)
                return v[:, dy:dy + HROWS, dx:dx + 80]

            # ---- conv1 ----
            h2t = {}
            for m in range(2):
                pt = ps1.tile([128, NTP], F32, tag="ps1")
                first = True
                for dy in range(3):
                    for dx in range(3):
                        t9 = dy * 3 + dx
                        wsl = slice(t9 * CHID + m * 128,
                                    t9 * CHID + m * 128 + 128)
                        nc.tensor.matmul(pt[:], w1[0][:, wsl], mk_rhs(xh, dy, dx),
                                         start=first, stop=False)
                        first = False
                        nc.tensor.matmul(pt[:], w1[1][:, wsl], mk_rhs(xl, dy, dx),
                                         start=False, stop=False)
                        nc.tensor.matmul(pt[:], w1[2][:, wsl], mk_rhs(xh2, dy, dx),
                                         start=False, stop=(dy == 2 and dx == 2))
                hhi = h2pool.tile([128, NTP], F16, tag=f"hhi{m}")
                nc.scalar.activation(hhi[:], pt[:], AT.Relu,
                                     bias=b1[:, m:m + 1], scale=s1[:, m:m + 1])
                a256 = h2pool.tile([128, NTP], F32, tag=f"a256{m}")
                nc.scalar.activation(a256[:], pt[:], AT.Relu,
                                     bias=b1[:, 2 + m:3 + m],
                                     scale=s1[:, 2 + m:3 + m])
                hlo = h2pool.tile([128, NTP], F16, tag=f"hlo{m}")
                nc.vector.scalar_tensor_tensor(hlo[:], hhi[:], -256.0, a256[:],
                                               op0=OP.mult, op1=OP.add)
                hhi2 = h2pool.tile([128, NTP], F16, tag=f"hhi2{m}")
                nc.vector.tensor_scalar_mul(hhi2[:], hhi[:], 1.0 / 256.0)
                h2t[m] = (hhi, hlo, hhi2)

            # ---- conv2 ----
            p2 = ps2.tile([COUT, NTP], F32, tag="ps2")
            for m in range(2):
                hhi, hlo, hhi2 = h2t[m]
                ws = slice(m * COUT, (m + 1) * COUT)
                nc.tensor.matmul(p2[:], w2[0][:, ws], hhi[:],
                                 start=(m == 0), stop=False)
                nc.tensor.matmul(p2[:], w2[1][:, ws], hlo[:],
                                 start=False, stop=False)
                nc.tensor.matmul(p2[:], w2[2][:, ws], hhi2[:],
                                 start=False, stop=(m == 1))
            nc.scalar.activation(lg[:, nt * NTP:(nt + 1) * NTP], p2[:],
                                 AT.Identity, bias=b2[:], scale=s2[:])

        nc.sync.dma_start(
            logits_out[b], lg[:].rearrange("c (h w) -> c h w", h=H, w=W))

        # ------------------- NMS chunks -------------------
        for k in range(CHUNKS):
            y0 = 120 * k - 4
            rcb = nmswork.tile([128, 640], F32, tag="big")
            nc.vector.memset(rcb[:], NEG)
            rcb_v = rcb[:].rearrange("p (r w) -> p r w", r=8, w=80)
            for r1 in range(8):
                p_lo = (r1 - y0) % 8
                h_lo = (y0 + p_lo) // 8
                if h_lo < 0:
                    h_lo += 1
                    p_lo += 8
                nh = min(60 - h_lo, (127 - p_lo) // 8 + 1)
                if nh <= 0:
                    continue
                src = (logits_out[b][8 * r1:8 * r1 + 8, h_lo:h_lo + nh, :]
                       .rearrange("r h w -> h r w"))
                a_lo = p_lo // 8
                p_mod = p_lo % 8
                dst = (rcb_v.rearrange("(a p8) r w -> a p8 r w", p8=8)
                       [a_lo:a_lo + nh, p_mod, :, :])
                nc.sync.dma_start(dst, src)

            rc = nmswork.tile([128, XT], F32, tag="big")
            nc.gpsimd.memset(rc[:], NEG)
            nc.gpsimd.tensor_copy(
                rc[:, 8:648].rearrange("p (w r) -> p w r", r=8, w=80),
                rcb[:].rearrange("p (r w) -> p w r", r=8, w=80))

            h3 = nmswork.tile([128, XT], F32, tag="big")
            nc.vector.tensor_max(h3[:, 1:655], rc[:, 0:654], rc[:, 2:656])
            nc.vector.tensor_max(h3[:, 1:655], h3[:, 1:655], rc[:, 1:655])
            h9 = nmswork.tile([128, XT], F32, tag="big")
            nc.vector.tensor_max(h9[:, 4:652], h3[:, 1:649], h3[:, 7:655])
            nc.vector.tensor_max(h9[:, 4:652], h9[:, 4:652], h3[:, 4:652])

            s1t = nmswork.tile([128, XT], F32, tag="big")
            nc.sync.dma_start(s1t[0:127, :], h9[1:128, :])
            s2t = nmswork.tile([128, XT], F32, tag="big")
            nc.sync.dma_start(s2t[1:128, :], h9[0:127, :])
            v3 = nmswork.tile([128, XT], F32, tag="big")
            nc.vector.tensor_max(v3[:, 4:652], s1t[:, 4:652], s2t[:, 4:652])
            nc.vector.tensor_max(v3[:, 4:652], v3[:, 4:652], h9[:, 4:652])
            s3p = nmswork.tile([128, XT], F32, tag="big")
            nc.sync.dma_start(s3p[0:125, :], v3[3:128, :])
            s3m = nmswork.tile([128, XT], F32, tag="big")
            nc.sync.dma_start(s3m[3:128, :], v3[0:125, :])
            pooled = nmswork.tile([128, XT], F32, tag="big")
            nc.vector.tensor_max(pooled[:, 4:652], s3p[:, 4:652], s3m[:, 4:652])
            nc.vector.tensor_max(pooled[:, 4:652], pooled[:, 4:652], v3[:, 4:652])

            mask = nmswork.tile([128, XT], F32, tag="big")
            nc.vector.tensor_tensor(mask[:, 4:652], rc[:, 4:652],
                                    pooled[:, 4:652], op=OP.is_equal)
            keep = keeppool.tile([128, XT], F32, tag="keep")
            nc.vector.tensor_mul(keep[:, 4:652], mask[:, 4:652], rc[:, 4:652])
            keep_tiles[(b, k)] = keep

            bmh = bmpool.tile([128, 128], F32, tag="bmh")
            nc.vector.tensor_reduce(
                out=bmh[:],
                in_=keep[:, 8:648].rearrange("p (b5 w5) -> p b5 w5", w5=5),
                axis=AX.X, op=OP.max)
            t1 = bmpool.tile([128, 128], F32, tag="t1")
            nc.sync.dma_start(t1[0:127, :], bmh[1:128, :])
            m2 = bmpool.tile([128, 128], F32, tag="m2")
            nc.vector.tensor_max(m2[0:127, :], bmh[0:127, :], t1[0:127, :])
            t2 = bmpool.tile([128, 128], F32, tag="t2")
            nc.sync.dma_start(t2[0:125, :], m2[2:127, :])
            m4 = bmpool.tile([128, 128], F32, tag="m4")
            nc.vector.tensor_max(m4[0:125, :], m2[0:125, :], t2[0:125, :])
            t3 = bmpool.tile([128, 128], F32, tag="t3")
            nc.sync.dma_start(t3[0:124, :], bmh[4:128, :])
            m5 = bmpool.tile([128, 128], F32, tag="m5")
            nc.vector.tensor_max(m5[0:124, :], m4[0:124, :], t3[0:124, :])
            nc.vector.tensor_scalar_max(m5[0:124, :], m5[0:124, :], 0.0)
            sel_rows = (m5[0:125, :]
                        .rearrange("(a p5) f -> a p5 f", p5=5)[0:24, 4, :])
            nc.sync.dma_start(cand[b][24 * k:24 * (k + 1), :], sel_rows)

            nc.sync.dma_start(prob_out[b, 120 * k:120 * k + 120, :],
                              rc[4:124, 8:648])

    # =====================================================================
    # exact top-1000 threshold per image
    # =====================================================================
    for b in range(B):
        for it in range(4):
            mx = m_all[:, b * 32 + it * 8: b * 32 + it * 8 + 8]
            nc.vector.max(out=mx, in_=cand[b][:])
            if it < 3:
                nc.vector.match_replace(out=cand[b][:], in_to_replace=mx,
                                        in_values=cand[b][:], imm_value=NEG)

    thr128 = selpool.tile([96, 128], F32, tag="thr128")
    for rnd in range(NSEL_ROUNDS):
        d4 = smpool.tile([1, 4], F32, tag="d4")
        nc.vector.tensor_sub(d4[:], hi4[:], lo4[:])
        thr_s = smpool.tile([1, 128], F32, tag="thr_s")
        v_thr = thr_s[:].rearrange("p (i t) -> p i t", i=4, t=TSEL)
        nc.vector.tensor_tensor(
            v_thr, frac[:].rearrange("p (i t) -> p i t", i=4, t=TSEL),
            d4[:].unsqueeze(2).to_broadcast([1, 4, TSEL]),
            op=OP.mult)
        nc.vector.tensor_tensor(
            v_thr, v_thr,
            lo4[:].unsqueeze(2).to_broadcast([1, 4, TSEL]),
            op=OP.add)
        pb = psb.tile([96, 128], F32, tag="psb")
        nc.tensor.matmul(pb[:], ones96[:], thr_s[:], start=True, stop=True)
        nc.scalar.copy(thr128[:], pb[:])

        cmps = selpool.tile([96, 4096], BF16, tag="cmps")
        nc.vector.tensor_tensor(
            cmps[:].rearrange("p (i t e) -> p i t e", i=4, t=TSEL, e=32),
            m_all[:].rearrange("p (i e) -> p i e", i=4).unsqueeze(2)
                    .to_broadcast([96, 4, TSEL, 32]),
            thr128[:].rearrange("p (i t) -> p i t", i=4).unsqueeze(3)
                     .to_broadcast([96, 4, TSEL, 32]),
            op=OP.is_ge)
        cm2 = selpool.tile([96, 128], F32, tag="cm2")
        nc.vector.tensor_reduce(
            out=cm2[:], in_=cmps[:].rearrange("p (f e) -> p f e", e=32),
            axis=AX.X, op=OP.add)
        cnts = smpool.tile([1, 128], F32, tag="cnts")
        nc.gpsimd.tensor_reduce(out=cnts[:], in_=cm2[:], axis=AX.C, op=OP.add)

        cge = smpool.tile([1, 128], F32, tag="cge")
        nc.vector.tensor_scalar(cge[:], cnts[:], float(TOPK), None,
                                op0=OP.is_ge)
        lom = smpool.tile([1, 128], F32, tag="lom")
        nc.vector.tensor_mul(lom[:], thr_s[:], cge[:])
        lo_new = smpool.tile([1, 4], F32, tag="lo_new")
        nc.vector.tensor_reduce(
            out=lo_new[:], in_=lom[:].rearrange("p (i t) -> p i t", i=4),
            axis=AX.X, op=OP.max)
        nc.vector.tensor_max(lo4[:], lo4[:], lo_new[:])
        him = smpool.tile([1, 128], F32, tag="him")
        nc.vector.scalar_tensor_tensor(him[:], cge[:], 1.0e30, thr_s[:],
                                       op0=OP.mult, op1=OP.add)
        hi_new = smpool.tile([1, 4], F32, tag="hi_new")
        nc.vector.tensor_reduce(
            out=hi_new[:], in_=him[:].rearrange("p (i t) -> p i t", i=4),
            axis=AX.X, op=OP.min)
        nc.vector.tensor_tensor(hi4[:], hi4[:], hi_new[:], op=OP.min)

    # exact v1000 = max{c in m_all : c < hi}
    hi_rep = smpool.tile([1, 128], F32, tag="hi_rep")
    nc.vector.tensor_copy(
        hi_rep[:].rearrange("p (i t) -> p i t", i=4, t=TSEL),
        hi4[:].unsqueeze(2).to_broadcast([1, 4, TSEL]))
    pbh = psb.tile([96, 128], F32, tag="psb")
    nc.tensor.matmul(pbh[:], ones96[:], hi_rep[:], start=True, stop=True)
    hi128 = selpool.tile([96, 128], F32, tag="hi128")
    nc.scalar.copy(hi128[:], pbh[:])
    maskv = selpool.tile([96, 128], F32, tag="maskv")
    nc.vector.tensor_tensor(maskv[:], m_all[:], hi128[:], op=OP.is_ge)
    nc.vector.tensor_scalar(maskv[:], maskv[:], -1.0, 1.0,
                            op0=OP.mult, op1=OP.add)  # 1 - ge  == (m < hi)
    mv = selpool.tile([96, 128], F32, tag="mv")
    nc.vector.tensor_mul(mv[:], maskv[:], m_all[:])
    vmax = smpool.tile([96, 4], F32, tag="vmax")
    nc.vector.tensor_reduce(
        out=vmax[:], in_=mv[:].rearrange("p (i e) -> p i e", i=4),
        axis=AX.X, op=OP.max)
    v4 = smpool.tile([1, 4], F32, tag="v4")
    nc.gpsimd.tensor_reduce(out=v4[:], in_=vmax[:], axis=AX.C, op=OP.max)
    pbt = psb.tile([128, 4], F32, tag="psb2")
    nc.tensor.matmul(pbt[:], ones128[:], v4[:], start=True, stop=True)
    nc.scalar.copy(thr_all[:], pbt[:])

    # =====================================================================
    # final thresholding + remaining outputs
    # =====================================================================
    for b in range(B):
        for k in range(CHUNKS):
            keep = keep_tiles[(b, k)]
            pnms = outpool.tile([128, 640], F32, tag="pnms")
            nc.vector.scalar_tensor_tensor(
                pnms[:], keep[:, 8:648], thr_all[:, b:b + 1], keep[:, 8:648],
                op0=OP.is_ge, op1=OP.mult)
            pred = outpool.tile([128, 640], I32, tag="pred")
            nc.gpsimd.tensor_scalar(pred[:], pnms[:], DET, None, op0=OP.is_ge)
            nc.sync.dma_start(pnms_out[b, 120 * k:120 * k + 120, :],
                              pnms[4:124, :])
            nc.sync.dma_start(pred_out[b, 120 * k:120 * k + 120, :],
                              pred[4:124, :])
    ctx.close()


def _host_prep(inputs):
    x = np.ascontiguousarray(inputs["x"], dtype=np.float32)
    f = lambda k: np.asarray(inputs[k], dtype=np.float32)
    wa, ba, ga, bta, ma, va = (f(k) for k in ("wa", "ba", "ga", "bta", "ma", "va"))
    wb, bb, gb, btb, mb, vb = (f(k) for k in ("wb", "bb", "gb", "btb", "mb", "vb"))

    EPS = np.float32(1e-5)
    inv1 = (ga / np.sqrt(va + EPS)).astype(np.float32)
    bias1 = ((ba - ma) * inv1 + bta).astype(np.float32)
    inv2 = (gb / np.sqrt(vb + EPS)).astype(np.float32)
    bias2 = ((bb - mb) * inv2 + btb).astype(np.float32)

    w1t = wa.transpose(2, 3, 1, 0).reshape(9, 128, 256).astype(np.float32)
    w1h, w1h2, w1l = _split_fp16(w1t)
    w2t = wb.reshape(COUT, 256).T.reshape(2, 128, COUT).astype(np.float32)
    w2h, w2h2, w2l = _split_fp16(w2t)

    fr = np.tile(np.arange(1, TSEL + 1, dtype=np.float32) / (TSEL + 1.0), 4)
    shared = {
        "w1h": w1h, "w1h2": w1h2, "w1l": w1l,
        "w2h": w2h, "w2h2": w2h2, "w2l": w2l,
        "sb1": np.stack([inv1, bias1]).astype(np.float32),
        "sb2": np.stack([inv2, bias2]).astype(np.float32),
        "frac": fr.reshape(1, 4 * TSEL).astype(np.float32),
    }
    return x, shared


def kernel(**inputs):
    x, shared = _host_prep(inputs)
    if "nc" not in _KERNEL_CACHE:
        _KERNEL_CACHE["nc"] = build_kernel()
    nc = _KERNEL_CACHE["nc"]

    in_maps = []
    for c in range(N_CORES):
        m = {"x": np.ascontiguousarray(x[c * B:(c + 1) * B])}
        m.update(shared)
        in_maps.append(m)
    res = run_bass_kernel_spmd(nc, in_maps, core_ids=list(range(N_CORES)))
    outs = res.results
    logits = np.concatenate([o["logits"] for o in outs], axis=0)
    prob = np.concatenate([o["prob"] for o in outs], axis=0)
    pnms = np.concatenate([o["pnms"] for o in outs], axis=0)
    pred = np.concatenate([o["pred"] for o in outs], axis=0)
    return logits, prob, pnms, pred
